# revision 30
# baseline (speedup 1.0000x reference)
"""AttentionWithRoPE on 8 trn2 NeuronCores.

Sharding (tensor-parallel over heads x data-parallel over batch):
  core c -> batch b = c // 4, head group g = c % 4 (heads [4g, 4g+4)).
Each core computes q/k/v projections for its 4 heads (columns
[512g, 512g+512) of Wq/Wk/Wv), causal attention with RoPE, and the
partial o_proj contribution  attn_out_local @ Wo[512g:512g+512, :].
The host gather sums the 4 partials per batch (row-parallel linear).

Design (306us/iter marginal on HW this epoch vs 439us for the prior
bf16 kernel and 392us for its prior-epoch stamp; absolute numbers
drift ~15% with tunnel/machine state — only compare within-run or
within-epoch):
- fp8(e4m3) DoubleRow q/k projections: hs and Wq/Wk are quantized
  host-side (weights pre-scaled x64, the 1/64 folded into host cos/sin
  for free), matmuls pair e-chunks [128,2,*] at 2x PE rate. v/o_proj
  and attention stay bf16 — numpy sweep: qk-fp8 1.69e-2 vs the 2e-2
  gate, while fp8 on v/av/o each blows it (3.3-4.2e-2). rel err on HW
  matches the sweep: 1.697e-2.
- causal-suffix trim (j>=1): diagonal score/AV/exp chunks compute only
  the valid q-suffix [128m:512); diagonals run m=3..0 so the AV
  accumulation group stops on a full-width matmul; mask multiplies
  shrink to the [128,128] triangular part. Numerically exact.
- everything else bf16 on-chip (wv/wo, qT/kT/v, exp, masks, output
  partials; host upcasts and sums); f32 PSUM.
- All four weight blocks + cos/sin fully SBUF-resident, loaded once at
  start with DMAs spread across the SP/ACT/Pool DGE queues; first two
  hs blocks prefetched on the ACT queue (cold-start 36us -> 14us).
- FUSED schedule: attention for block j pumps whole projection units
  of block j+1 (16-matmul q/k groups, 2-subtile v half-passes) between
  its score/AV chains, so the PE has dense work while exp/mask
  latencies resolve. Block 3 (no proj filler) instead gets deepened
  score/AV pipelines from the banks freed by the projection pools.
- o_proj additionally software-pipelined one block behind attention in
  4-matmul subgroups (OProj), interleaved into the kb stream; for the
  last block they are spread across the 16-kb streams with one saved
  for each head's normalization tail (the only gap left there).
- RoPE rotate-half via SBUF->SBUF DMA on bf16 tiles (engines cannot
  cross partitions), cos/sin multiplies on DVE.
- Softmax denominator: bf16 partial sums on DVE (two interleaved
  accumulators), then gpsimd(Pool) partition_all_reduce + DVE
  reciprocal — the PE is entirely out of the normalization path.
- PSUM: proj pps 2 + vps 2 (scoped to blocks 0-2), attn scp 2 + avp 1
  + opp 1; after the proj pools close, block 3 and the final o_proj
  drain get scp3 2 + avp3 1 + opp3 1 from the freed banks (o_proj
  subgroups alternate opp/opp3 so evictions pipeline).
Known-bad variants (tried, slower on HW despite better sim times):
exp on [128,1024] pairs (chain latency); h-major o_proj n-pairs with
two interleaved open PSUM accumulation groups; deferred normalization
chain; single orow buffer (OUT-DMA latency serializes o_proj); halved
hs DMAs (extra transfer overhead, sim +6us); filler pacing pos%3
(sim-neutral); hs+hs8 both on gpsimd queue with OUT on ACT queue
(serializes the two hs streams, HW 333us vs 306us).
"""

import sys

for _p in ("/opt/trn_rl_repo", "/root/.axon_site/_ro/trn_rl_repo"):
    if _p not in sys.path:
        sys.path.insert(0, _p)

import ml_dtypes
import numpy as np

import concourse.bass as bass
import concourse.tile as tile
from concourse import bacc, mybir
from concourse.bass_utils import run_bass_kernel_spmd

f32 = mybir.dt.float32
bf16 = mybir.dt.bfloat16
fp8 = mybir.dt.float8e4
DR = mybir.MatmulPerfMode.DoubleRow
EXP = mybir.ActivationFunctionType.Exp
COPY = mybir.ActivationFunctionType.Copy

B = 2
S = 2048
E = 2048
D = 128
HL = 4          # local heads per core
EL = HL * D     # 512, local projection width
NB = S // 512   # 4 query/key 512-blocks
EC = E // 128   # 16 contraction chunks
SCALE = float(1.0 / np.sqrt(D))

_CACHE = {}
LAST_EXEC_NS = None


def _build(repeat=1):
    from contextlib import ExitStack

    nc = bacc.Bacc("TRN2", target_bir_lowering=False, debug=False, num_devices=8)

    HST = nc.dram_tensor("hsT", [E, S], bf16, kind="ExternalInput")
    HS8T = nc.dram_tensor("hs8T", [E, S], fp8, kind="ExternalInput")
    WQ = nc.dram_tensor("wq", [E, EL], fp8, kind="ExternalInput")
    WK = nc.dram_tensor("wk", [E, EL], fp8, kind="ExternalInput")
    WV = nc.dram_tensor("wv", [E, EL], bf16, kind="ExternalInput")
    WO = nc.dram_tensor("wo", [EL, E], bf16, kind="ExternalInput")
    COS = nc.dram_tensor("cosT", [D, S], bf16, kind="ExternalInput")
    SIN = nc.dram_tensor("sinTs", [D, S], bf16, kind="ExternalInput")  # sign-folded
    MSK = nc.dram_tensor("masks", [128, 4, 512], bf16, kind="ExternalInput")
    OUT = nc.dram_tensor("out", [S, E], bf16, kind="ExternalOutput")

    with tile.TileContext(nc) as tc, nc.allow_low_precision("bf16 compute by design"):
        with ExitStack() as octx:
            # kernel-lifetime residents, per-partition KB:
            #   wq/wk/wv 16 each, wo 16, kT/qT/v 16 each, cos/sin 8, masks 4
            res = octx.enter_context(tc.tile_pool(name="res", bufs=1))
            wq_sb = res.tile([128, EC, EL], fp8, tag="wq")
            wk_sb = res.tile([128, EC, EL], fp8, tag="wk")
            wv_sb = res.tile([128, EC, EL], bf16, tag="wv")
            wo_sb = res.tile([128, HL, E], bf16, tag="wo")
            kT = [res.tile([128, S], bf16, tag=f"kT{h}", name=f"kT{h}") for h in range(HL)]
            qT = [res.tile([128, S], bf16, tag=f"qT{h}", name=f"qT{h}") for h in range(HL)]
            v_sb = res.tile([128, NB * 4, EL], bf16, tag="v")
            cos_sb = res.tile([128, S], bf16, tag="cos")
            sin_sb = res.tile([128, S], bf16, tag="sin")
            masks = res.tile([128, 4, 512], bf16, tag="masks")
            # spread the startup loads across engine DGE queues so they run
            # in parallel; j=0 needs wv (then wq/wk) as early as possible
            nc.sync.dma_start(wv_sb[:], WV[:].rearrange("(c p) m -> p c m", p=128))
            nc.sync.dma_start(wq_sb[:], WQ[:].rearrange("(c p) m -> p c m", p=128))
            nc.gpsimd.dma_start(wk_sb[:], WK[:].rearrange("(c p) m -> p c m", p=128))
            nc.gpsimd.dma_start(cos_sb[:], COS[:])
            nc.gpsimd.dma_start(sin_sb[:], SIN[:])
            nc.gpsimd.dma_start(masks[:], MSK[:])
            nc.gpsimd.dma_start(wo_sb[:], WO[:].rearrange("(c p) m -> p c m", p=128))

            for _rep in range(repeat):
                # fused projection + attention: attention for block j pumps
                # whole projection units (16-matmul q/k groups, v half-passes)
                # of block j+1 between its score/AV chains, so the PE always
                # has dense work while exp/mask latencies resolve
                with ExitStack() as ctx:
                    hsp = ctx.enter_context(tc.tile_pool(name="hs1", bufs=2))
                    tmp = ctx.enter_context(tc.tile_pool(name="tmp1", bufs=2))
                    sbp = ctx.enter_context(tc.tile_pool(name="sb2", bufs=6))
                    onp = ctx.enter_context(tc.tile_pool(name="on2", bufs=8))
                    bcp = ctx.enter_context(tc.tile_pool(name="bc2", bufs=1))
                    lap = ctx.enter_context(tc.tile_pool(name="la2", bufs=4))
                    orp = ctx.enter_context(tc.tile_pool(name="or2", bufs=3))
                    scp = ctx.enter_context(tc.tile_pool(name="scps", bufs=2, space="PSUM"))
                    avp = ctx.enter_context(tc.tile_pool(name="avps", bufs=1, space="PSUM"))
                    opp = ctx.enter_context(tc.tile_pool(name="opps", bufs=1, space="PSUM"))
                    psum_extra = {}

                    hs_pre = {}

                    def load_hs(j, eng, eng8=None):
                        t = hsp.tile([128, EC, 512], bf16, tag="hscol",
                                     name=f"hs{_rep}_{j}")
                        eng.dma_start(
                            t[:],
                            HST[:, j * 512:(j + 1) * 512].rearrange(
                                "(c p) s -> p c s", p=128
                            ),
                        )
                        t8 = hsp.tile([128, EC, 512], fp8, tag="hs8col",
                                      name=f"hs8{_rep}_{j}")
                        (eng8 or eng).dma_start(
                            t8[:],
                            HS8T[:, j * 512:(j + 1) * 512].rearrange(
                                "(c p) s -> p c s", p=128
                            ),
                        )
                        return t, t8

                    hs_pre[0] = load_hs(0, nc.scalar)
                    hs_pre[1] = load_hs(1, nc.scalar)

                    def rope_evict(dst, ps, cos_t, sin_t):
                        # dst = raw*cosT + rot(raw)*sinT_signed
                        raw = tmp.tile([128, 512], bf16, tag="qkraw")
                        nc.scalar.activation(raw[:], ps[:], COPY)
                        rot = tmp.tile([128, 512], bf16, tag="qkrot")
                        nc.sync.dma_start(rot[0:64, :], raw[64:128, :])
                        nc.sync.dma_start(rot[64:128, :], raw[0:64, :])
                        t1 = tmp.tile([128, 512], bf16, tag="ropet1")
                        nc.vector.tensor_mul(t1[:], raw[:], cos_t)
                        nc.vector.tensor_mul(dst, rot[:], sin_t)
                        nc.vector.tensor_add(dst, dst, t1[:])

                    def proj_units(j):
                        """Emission closures for block j's projections: two
                        2-subtile v passes + a q and a k unit per head."""
                        sl = slice(j * 512, (j + 1) * 512)
                        hs_t, hs8_t = (hs_pre.pop(j) if j in hs_pre
                                       else load_hs(j, nc.sync, nc.gpsimd))

                        def v_pass(p):
                            def emit():
                                vp = [
                                    psum_extra["vps"].tile([128, EL], f32, tag="vps",
                                             name=f"vp{_rep}_{j}_{p}_{t}")
                                    for t in range(2)
                                ]
                                for e in range(EC):
                                    for t in range(2):
                                        i = 2 * p + t
                                        nc.tensor.matmul(
                                            vp[t][:],
                                            hs_t[:, e, i * 128:(i + 1) * 128],
                                            wv_sb[:, e, :],
                                            start=(e == 0),
                                            stop=(e == EC - 1),
                                        )
                                for t in range(2):
                                    i = 2 * p + t
                                    nc.scalar.activation(
                                        v_sb[:, j * 4 + i, :], vp[t][:], COPY
                                    )
                            return emit

                        def qk_unit(h, which):
                            w_sb, dstT = (wq_sb, qT) if which == "q" else (wk_sb, kT)

                            def emit():
                                ps = psum_extra["pps"].tile([128, 512], f32, tag="qkps")
                                for e in range(0, EC, 2):
                                    nc.tensor.matmul(
                                        ps[:],
                                        w_sb[:, e:e + 2, h * 128:(h + 1) * 128],
                                        hs8_t[:, e:e + 2, :],
                                        start=(e == 0),
                                        stop=(e == EC - 2),
                                        perf_mode=DR,
                                    )
                                rope_evict(dstT[h][:, sl], ps,
                                           cos_sb[:, sl], sin_sb[:, sl])
                            return emit

                        units = [v_pass(0), v_pass(1)]
                        for h in range(HL):
                            units.append(qk_unit(h, "q"))
                            units.append(qk_unit(h, "k"))
                        return units

                    class OProj:
                        """One output row-block of o_proj, emitted in four
                        4-matmul subgroups interleaved into the kb stream."""

                        def __init__(self, jj, i, o_n):
                            self.jj, self.i, self.o_n = jj, i, o_n
                            self.n = 0
                            self.orow = orp.tile([128, E], bf16, tag="orow",
                                                 name=f"orow{_rep}_{jj}_{i}")

                        def emit_subgroup(self):
                            if self.n >= 4:
                                return
                            n, i, jj = self.n, self.i, self.jj
                            pool = (psum_extra["opp3"]
                                    if "opp3" in psum_extra and n % 2
                                    else opp)
                            op_ps = pool.tile([128, 512], f32, tag="op",
                                              name=f"op{_rep}_{jj}_{i}_{n}")
                            for h in range(HL):
                                nc.tensor.matmul(
                                    op_ps[:],
                                    self.o_n[h][:, i * 128:(i + 1) * 128],
                                    wo_sb[:, h, n * 512:(n + 1) * 512],
                                    start=(h == 0),
                                    stop=(h == HL - 1),
                                )
                            nc.vector.tensor_copy(
                                self.orow[:, n * 512:(n + 1) * 512], op_ps[:]
                            )
                            self.n += 1
                            if self.n == 4:
                                # final-block OUT writes go on gpsimd (done
                                # loading by then) so the NEXT rep's early
                                # rope DMAs on sync don't queue behind 2MB
                                eng = nc.gpsimd if self.jj == NB - 1 else nc.sync
                                eng.dma_start(
                                    OUT[jj * 512 + i * 128:jj * 512 + (i + 1) * 128, :],
                                    self.orow[:],
                                )

                        def finish(self):
                            while self.n < 4:
                                self.emit_subgroup()

                    norm_ctr = [0]

                    def emit_norm(av_ps, lacc, o_norm, h):
                        # l = allreduce_over_partitions(lacc0 + lacc1); the
                        # whole chain runs on Pool + DVE, no PE involvement
                        lsum = bcp.tile([128, 512], f32, tag="lsum")
                        nc.vector.tensor_add(lsum[:], lacc[0][:], lacc[1][:])
                        lred = bcp.tile([128, 512], f32, tag="lred")
                        nc.gpsimd.partition_all_reduce(
                            lred[:], lsum[:], channels=128,
                            reduce_op=bass.bass_isa.ReduceOp.add,
                        )
                        bc_sb = bcp.tile([128, 512], f32, tag="bcsb")
                        nc.vector.reciprocal(bc_sb[:], lred[:])
                        norm_ctr[0] += 1
                        on = onp.tile([128, 512], bf16, tag="onorm",
                                      name=f"on{_rep}_{norm_ctr[0]}_{h}")
                        nc.vector.tensor_mul(on[:], av_ps[:], bc_sb[:])
                        o_norm[h] = on

                    def attn_block(j, filler, o_prev):
                        sl = slice(j * 512, (j + 1) * 512)
                        nkb = 4 * j + 4
                        # causal trim (j>=1): diagonal chunk m only touches
                        # queries q >= 128m, so compute the [128m:512) suffix
                        # only. Diagonals run m=3..0 so the group's stop lands
                        # on a full-width matmul (m=0); j=0 keeps full width
                        # (its pos<2 lacc copies must cover all 512 cols).
                        if j >= 1:
                            order = list(range(4 * j)) + [4 * j + 3, 4 * j + 2,
                                                          4 * j + 1, 4 * j]
                        else:
                            order = list(range(nkb))
                        o_norm = [None] * HL
                        for h in range(HL):
                            pending = OProj(j - 1, h, o_prev) if o_prev else None
                            if "avp3" in psum_extra and h % 2:
                                av_ps = psum_extra["avp3"].tile(
                                    [128, 512], f32, tag="av")
                            else:
                                av_ps = avp.tile([128, 512], f32, tag="av")
                            lacc = [
                                lap.tile([128, 512], bf16, tag="lacc",
                                         name=f"lacc{_rep}_{j}_{h}_{kk}")
                                for kk in range(2)
                            ]
                            for pos, kb in enumerate(order):
                                m = kb - 4 * j
                                qs = 128 * m if (j >= 1 and m > 0) else 0
                                if "scp3" in psum_extra and pos % 2:
                                    sc_ps = psum_extra["scp3"].tile(
                                        [128, 512], f32, tag="sc")
                                else:
                                    sc_ps = scp.tile([128, 512], f32, tag="sc")
                                nc.tensor.matmul(
                                    sc_ps[:, qs:],
                                    kT[h][:, kb * 128:(kb + 1) * 128],
                                    qT[h][:, j * 512 + qs:(j + 1) * 512],
                                    start=True,
                                    stop=True,
                                )
                                ex = sbp.tile([128, 512], bf16, tag="expT")
                                nc.scalar.activation(ex[:, qs:], sc_ps[:, qs:],
                                                     EXP, scale=SCALE)
                                if m >= 0:  # diagonal: causal mask
                                    if qs > 0 or m == 0:
                                        # triangular part only spans 128 cols
                                        nc.vector.tensor_mul(
                                            ex[:, qs:qs + 128],
                                            ex[:, qs:qs + 128],
                                            masks[:, m, qs:qs + 128],
                                        )
                                    else:  # j == 0, m > 0: untrimmed
                                        nc.vector.tensor_mul(
                                            ex[:], ex[:], masks[:, m, :]
                                        )
                                nc.tensor.matmul(
                                    av_ps[:, qs:],
                                    v_sb[:, kb, h * 128:(h + 1) * 128],
                                    ex[:, qs:],
                                    start=(pos == 0),
                                    stop=(pos == nkb - 1),
                                )
                                if pos < 2:
                                    nc.vector.tensor_copy(lacc[pos][:], ex[:])
                                else:
                                    nc.vector.tensor_add(
                                        lacc[pos % 2][:, qs:],
                                        lacc[pos % 2][:, qs:],
                                        ex[:, qs:],
                                    )
                                # o_proj filler pacing: for the last block
                                # (no proj filler) spread subgroups across the
                                # long kb stream and keep one for the norm tail
                                pace = 5 if j == NB - 1 else 1
                                if pending is not None and (pos - 1) % pace == 0 and pos >= 1:
                                    pending.emit_subgroup()
                                if pos % 4 == 2 and filler:
                                    filler.pop(0)()
                            if j < NB - 1 and pending is not None:
                                pending.finish()
                            emit_norm(av_ps, lacc, o_norm, h)
                            if pending is not None:
                                pending.finish()
                            if filler:
                                filler.pop(0)()
                        for u in filler:
                            u()
                        return o_norm

                    # j=0..2 fused with projections of j+1; the proj PSUM
                    # pools live only for this span
                    with ExitStack() as pctx:
                        pps = pctx.enter_context(
                            tc.tile_pool(name="pps1", bufs=2, space="PSUM"))
                        vps = pctx.enter_context(
                            tc.tile_pool(name="vps1", bufs=2, space="PSUM"))
                        psum_extra["pps"] = pps
                        psum_extra["vps"] = vps
                        for u in proj_units(0):
                            u()
                        o_prev = None
                        for j in range(NB - 1):
                            o_prev = attn_block(j, proj_units(j + 1), o_prev)
                    del psum_extra["pps"], psum_extra["vps"]
                    # j=3 has no proj filler; deepen its pipelines with the
                    # banks freed by the projection pools
                    psum_extra["scp3"] = ctx.enter_context(
                        tc.tile_pool(name="scp3", bufs=2, space="PSUM"))
                    psum_extra["avp3"] = ctx.enter_context(
                        tc.tile_pool(name="avp3", bufs=1, space="PSUM"))
                    psum_extra["opp3"] = ctx.enter_context(
                        tc.tile_pool(name="opp3", bufs=1, space="PSUM"))
                    o_prev = attn_block(NB - 1, [], o_prev)

                    for i in range(4):
                        OProj(NB - 1, i, o_prev).finish()

    nc.compile()
    return nc


def _get_nc(repeat=1):
    key = ("nc", repeat)
    if key not in _CACHE:
        _CACHE[key] = _build(repeat=repeat)
    return _CACHE[key]


def _make_masks():
    sk = np.arange(128)[:, None]
    sq = np.arange(512)[None, :]
    m = np.stack([(sq >= sk + 128 * mm) for mm in range(4)], axis=1)
    return m.astype(ml_dtypes.bfloat16)


def _prepare_in_maps(hidden_states, cos, sin, Wq, Wk, Wv, Wo):
    bf = ml_dtypes.bfloat16
    hidden_states = np.asarray(hidden_states, dtype=np.float32)
    cos = np.asarray(cos, dtype=np.float32)
    sin = np.asarray(sin, dtype=np.float32)

    masks = _make_masks()
    e4 = ml_dtypes.float8_e4m3
    WS = 64.0  # power-of-2 fp8 weight scale, folded into cos/sin
    in_maps = []
    hsT = [np.ascontiguousarray(hidden_states[b].T).astype(bf) for b in range(B)]
    hs8T = [np.clip(np.ascontiguousarray(hidden_states[b].T), -240, 240).astype(e4)
            for b in range(B)]
    cosT = [np.ascontiguousarray(cos[b].T / WS).astype(bf) for b in range(B)]
    sinTs = []
    for b in range(B):
        s = np.ascontiguousarray(sin[b].T) / WS
        s[:64] *= -1.0
        sinTs.append(s.astype(bf))
    Wq8 = np.clip(np.asarray(Wq, dtype=np.float32) * WS, -240, 240).astype(e4)
    Wk8 = np.clip(np.asarray(Wk, dtype=np.float32) * WS, -240, 240).astype(e4)
    Wv = np.asarray(Wv, dtype=np.float32).astype(bf)
    Wo = np.asarray(Wo, dtype=np.float32).astype(bf)
    for c in range(8):
        b, g = c // 4, c % 4
        cols = slice(512 * g, 512 * (g + 1))
        in_maps.append({
            "hsT": hsT[b],
            "hs8T": hs8T[b],
            "wq": np.ascontiguousarray(Wq8[:, cols]),
            "wk": np.ascontiguousarray(Wk8[:, cols]),
            "wv": np.ascontiguousarray(Wv[:, cols]),
            "wo": np.ascontiguousarray(Wo[cols, :]),
            "cosT": cosT[b],
            "sinTs": sinTs[b],
            "masks": masks,
        })
    return in_maps


def kernel(hidden_states, cos, sin, Wq, Wk, Wv, Wo):
    nc = _get_nc()
    in_maps = _prepare_in_maps(hidden_states, cos, sin, Wq, Wk, Wv, Wo)
    res = run_bass_kernel_spmd(nc, in_maps, core_ids=list(range(8)))
    global LAST_EXEC_NS
    if res.exec_time_ns is not None:
        LAST_EXEC_NS = res.exec_time_ns
    out = np.empty((B, S, E), dtype=np.float32)
    for b in range(B):
        acc = res.results[4 * b]["out"].astype(np.float32)
        for g in range(1, 4):
            acc = acc + res.results[4 * b + g]["out"]
        out[b] = acc
    return out



# revision 32
# speedup vs baseline: 1.2694x; 1.2694x over previous
"""AttentionWithRoPE on 8 trn2 NeuronCores.

Sharding (tensor-parallel over heads x data-parallel over batch):
  core c -> batch b = c // 4, head group g = c % 4 (heads [4g, 4g+4)).
Each core computes q/k/v projections for its 4 heads (columns
[512g, 512g+512) of Wq/Wk/Wv), causal attention with RoPE, and the
partial o_proj contribution  attn_out_local @ Wo[512g:512g+512, :].
The host gather sums the 4 partials per batch (row-parallel linear).

Design (306us/iter marginal on HW this epoch vs 439us for the prior
bf16 kernel and 392us for its prior-epoch stamp; absolute numbers
drift ~15% with tunnel/machine state — only compare within-run or
within-epoch):
- fp8(e4m3) DoubleRow q/k projections: hs and Wq/Wk are quantized
  host-side (weights pre-scaled x64, the 1/64 folded into host cos/sin
  for free), matmuls pair e-chunks [128,2,*] at 2x PE rate. v/o_proj
  and attention stay bf16 — numpy sweep: qk-fp8 1.69e-2 vs the 2e-2
  gate, while fp8 on v/av/o each blows it (3.3-4.2e-2). rel err on HW
  matches the sweep: 1.697e-2.
- causal-suffix trim (j>=1): diagonal score/AV/exp chunks compute only
  the valid q-suffix [128m:512); diagonals run m=3..0 so the AV
  accumulation group stops on a full-width matmul; mask multiplies
  shrink to the [128,128] triangular part. Numerically exact.
- everything else bf16 on-chip (wv/wo, qT/kT/v, exp, masks, output
  partials; host upcasts and sums); f32 PSUM.
- All four weight blocks + cos/sin fully SBUF-resident, loaded once at
  start with DMAs spread across the SP/ACT/Pool DGE queues; first two
  hs blocks prefetched on the ACT queue (cold-start 36us -> 14us).
- FUSED schedule: attention for block j pumps whole projection units
  of block j+1 (16-matmul q/k groups, 2-subtile v half-passes) between
  its score/AV chains, so the PE has dense work while exp/mask
  latencies resolve. Block 3 (no proj filler) instead gets deepened
  score/AV pipelines from the banks freed by the projection pools.
- o_proj additionally software-pipelined one block behind attention in
  4-matmul subgroups (OProj), interleaved into the kb stream; for the
  last block they are spread across the 16-kb streams with one saved
  for each head's normalization tail (the only gap left there).
- RoPE rotate-half via SBUF->SBUF DMA on bf16 tiles (engines cannot
  cross partitions), cos/sin multiplies on DVE.
- Softmax denominator: bf16 partial sums on DVE (two interleaved
  accumulators), then gpsimd(Pool) partition_all_reduce + DVE
  reciprocal — the PE is entirely out of the normalization path.
- PSUM: proj pps 2 + vps 2 (scoped to blocks 0-2), attn scp 2 + avp 1
  + opp 1; after the proj pools close, block 3 and the final o_proj
  drain get scp3 2 + avp3 1 + opp3 1 from the freed banks (o_proj
  subgroups alternate opp/opp3 so evictions pipeline).
Known-bad variants (tried, slower on HW despite better sim times):
exp on [128,1024] pairs (chain latency); h-major o_proj n-pairs with
two interleaved open PSUM accumulation groups; deferred normalization
chain; single orow buffer (OUT-DMA latency serializes o_proj); halved
hs DMAs (extra transfer overhead, sim +6us); filler pacing pos%3
(sim-neutral); hs+hs8 both on gpsimd queue with OUT on ACT queue
(serializes the two hs streams, HW 333us vs 306us).
"""

import sys

for _p in ("/opt/trn_rl_repo", "/root/.axon_site/_ro/trn_rl_repo"):
    if _p not in sys.path:
        sys.path.insert(0, _p)

import ml_dtypes
import numpy as np

import concourse.bass as bass
import concourse.tile as tile
from concourse import bacc, mybir
from concourse.bass_utils import run_bass_kernel_spmd

f32 = mybir.dt.float32
bf16 = mybir.dt.bfloat16
fp8 = mybir.dt.float8e4
DR = mybir.MatmulPerfMode.DoubleRow
EXP = mybir.ActivationFunctionType.Exp
COPY = mybir.ActivationFunctionType.Copy

B = 2
S = 2048
E = 2048
D = 128
HL = 4          # local heads per core
EL = HL * D     # 512, local projection width
NB = S // 512   # 4 query/key 512-blocks
EC = E // 128   # 16 contraction chunks
SCALE = float(1.0 / np.sqrt(D))

_CACHE = {}
LAST_EXEC_NS = None


def _build(repeat=1):
    from contextlib import ExitStack

    nc = bacc.Bacc("TRN2", target_bir_lowering=False, debug=False, num_devices=8)

    HST = nc.dram_tensor("hsT", [E, S], bf16, kind="ExternalInput")
    HS8T = nc.dram_tensor("hs8T", [E, S], fp8, kind="ExternalInput")
    WQ = nc.dram_tensor("wq", [E, EL], fp8, kind="ExternalInput")
    WK = nc.dram_tensor("wk", [E, EL], fp8, kind="ExternalInput")
    WV = nc.dram_tensor("wv", [E, EL], bf16, kind="ExternalInput")
    WO = nc.dram_tensor("wo", [EL, E], bf16, kind="ExternalInput")
    COS = nc.dram_tensor("cosT", [D, S], bf16, kind="ExternalInput")
    SIN = nc.dram_tensor("sinTs", [D, S], bf16, kind="ExternalInput")  # sign-folded
    MSK = nc.dram_tensor("masks", [128, 4, 512], bf16, kind="ExternalInput")
    OUT = nc.dram_tensor("out", [S, E], bf16, kind="ExternalOutput")

    with tile.TileContext(nc) as tc, nc.allow_low_precision("bf16 compute by design"):
        with ExitStack() as octx:
            # kernel-lifetime residents, per-partition KB:
            #   wq/wk/wv 16 each, wo 16, kT/qT/v 16 each, cos/sin 8, masks 4
            res = octx.enter_context(tc.tile_pool(name="res", bufs=1))
            wq_sb = res.tile([128, EC, EL], fp8, tag="wq")
            wk_sb = res.tile([128, EC, EL], fp8, tag="wk")
            wv_sb = res.tile([128, EC, EL], bf16, tag="wv")
            wo_sb = res.tile([128, HL, E], bf16, tag="wo")
            kT = [res.tile([128, S], bf16, tag=f"kT{h}", name=f"kT{h}") for h in range(HL)]
            qT = [res.tile([128, S], bf16, tag=f"qT{h}", name=f"qT{h}") for h in range(HL)]
            v_sb = res.tile([128, NB * 4, EL], bf16, tag="v")
            cos_sb = res.tile([128, S], bf16, tag="cos")
            sin_sb = res.tile([128, S], bf16, tag="sin")
            masks = res.tile([128, 4, 512], bf16, tag="masks")
            # spread the startup loads across engine DGE queues so they run
            # in parallel; j=0 needs wv (then wq/wk) as early as possible
            nc.sync.dma_start(wv_sb[:], WV[:].rearrange("(c p) m -> p c m", p=128))
            nc.sync.dma_start(wq_sb[:], WQ[:].rearrange("(c p) m -> p c m", p=128))
            nc.gpsimd.dma_start(wk_sb[:], WK[:].rearrange("(c p) m -> p c m", p=128))
            nc.gpsimd.dma_start(cos_sb[:], COS[:])
            nc.gpsimd.dma_start(sin_sb[:], SIN[:])
            nc.gpsimd.dma_start(masks[:], MSK[:])
            nc.gpsimd.dma_start(wo_sb[:], WO[:].rearrange("(c p) m -> p c m", p=128))

            for _rep in range(repeat):
                # fused projection + attention: attention for block j pumps
                # whole projection units (16-matmul q/k groups, v half-passes)
                # of block j+1 between its score/AV chains, so the PE always
                # has dense work while exp/mask latencies resolve
                with ExitStack() as ctx:
                    hsp = ctx.enter_context(tc.tile_pool(name="hs1", bufs=2))
                    tmp = ctx.enter_context(tc.tile_pool(name="tmp1", bufs=2))
                    sbp = ctx.enter_context(tc.tile_pool(name="sb2", bufs=4))
                    onp = ctx.enter_context(tc.tile_pool(name="on2", bufs=8))
                    bcp = ctx.enter_context(tc.tile_pool(name="bc2", bufs=1))
                    lap = ctx.enter_context(tc.tile_pool(name="la2", bufs=4))
                    orp = ctx.enter_context(tc.tile_pool(name="or2", bufs=2))
                    scp = ctx.enter_context(tc.tile_pool(name="scps", bufs=2, space="PSUM"))
                    avp = ctx.enter_context(tc.tile_pool(name="avps", bufs=1, space="PSUM"))
                    opp = ctx.enter_context(tc.tile_pool(name="opps", bufs=1, space="PSUM"))
                    psum_extra = {}

                    hs_pre = {}

                    def load_hs(j, eng, eng8=None):
                        t = hsp.tile([128, EC, 512], bf16, tag="hscol",
                                     name=f"hs{_rep}_{j}")
                        eng.dma_start(
                            t[:],
                            HST[:, j * 512:(j + 1) * 512].rearrange(
                                "(c p) s -> p c s", p=128
                            ),
                        )
                        t8 = hsp.tile([128, EC, 512], fp8, tag="hs8col",
                                      name=f"hs8{_rep}_{j}")
                        (eng8 or eng).dma_start(
                            t8[:],
                            HS8T[:, j * 512:(j + 1) * 512].rearrange(
                                "(c p) s -> p c s", p=128
                            ),
                        )
                        return t, t8

                    hs_pre[0] = load_hs(0, nc.scalar)
                    hs_pre[1] = load_hs(1, nc.scalar)

                    def rope_evict(dst, ps, cos_t, sin_t):
                        # dst = raw*cosT + rot(raw)*sinT_signed
                        raw = tmp.tile([128, 512], bf16, tag="qkraw")
                        nc.scalar.activation(raw[:], ps[:], COPY)
                        rot = tmp.tile([128, 512], bf16, tag="qkrot")
                        nc.sync.dma_start(rot[0:64, :], raw[64:128, :])
                        nc.sync.dma_start(rot[64:128, :], raw[0:64, :])
                        t1 = tmp.tile([128, 512], bf16, tag="ropet1")
                        nc.vector.tensor_mul(t1[:], raw[:], cos_t)
                        nc.vector.tensor_mul(dst, rot[:], sin_t)
                        nc.vector.tensor_add(dst, dst, t1[:])

                    def proj_units(j):
                        """Emission closures for block j's projections: two
                        2-subtile v passes + a q and a k unit per head."""
                        sl = slice(j * 512, (j + 1) * 512)
                        hs_t, hs8_t = (hs_pre.pop(j) if j in hs_pre
                                       else load_hs(j, nc.sync, nc.gpsimd))

                        def v_pass(p):
                            def emit():
                                vp = [
                                    psum_extra["vps"].tile([128, EL], f32, tag="vps",
                                             name=f"vp{_rep}_{j}_{p}_{t}")
                                    for t in range(2)
                                ]
                                for e in range(EC):
                                    for t in range(2):
                                        i = 2 * p + t
                                        nc.tensor.matmul(
                                            vp[t][:],
                                            hs_t[:, e, i * 128:(i + 1) * 128],
                                            wv_sb[:, e, :],
                                            start=(e == 0),
                                            stop=(e == EC - 1),
                                        )
                                for t in range(2):
                                    i = 2 * p + t
                                    nc.scalar.activation(
                                        v_sb[:, j * 4 + i, :], vp[t][:], COPY
                                    )
                            return emit

                        def qk_unit(h, which):
                            w_sb, dstT = (wq_sb, qT) if which == "q" else (wk_sb, kT)

                            def emit():
                                ps = psum_extra["pps"].tile([128, 512], f32, tag="qkps")
                                for e in range(0, EC, 2):
                                    nc.tensor.matmul(
                                        ps[:],
                                        w_sb[:, e:e + 2, h * 128:(h + 1) * 128],
                                        hs8_t[:, e:e + 2, :],
                                        start=(e == 0),
                                        stop=(e == EC - 2),
                                        perf_mode=DR,
                                    )
                                rope_evict(dstT[h][:, sl], ps,
                                           cos_sb[:, sl], sin_sb[:, sl])
                            return emit

                        units = [v_pass(0), v_pass(1)]
                        for h in range(HL):
                            units.append(qk_unit(h, "q"))
                            units.append(qk_unit(h, "k"))
                        return units

                    class OProj:
                        """One output row-block of o_proj, emitted in four
                        4-matmul subgroups interleaved into the kb stream."""

                        def __init__(self, jj, i, o_n):
                            self.jj, self.i, self.o_n = jj, i, o_n
                            self.n = 0
                            self.orow = orp.tile([128, E], bf16, tag="orow",
                                                 name=f"orow{_rep}_{jj}_{i}")

                        def emit_subgroup(self):
                            if self.n >= 4:
                                return
                            n, i, jj = self.n, self.i, self.jj
                            pool = (psum_extra["opp3"]
                                    if "opp3" in psum_extra and n % 2
                                    else opp)
                            op_ps = pool.tile([128, 512], f32, tag="op",
                                              name=f"op{_rep}_{jj}_{i}_{n}")
                            for h in range(HL):
                                nc.tensor.matmul(
                                    op_ps[:],
                                    self.o_n[h][:, i * 128:(i + 1) * 128],
                                    wo_sb[:, h, n * 512:(n + 1) * 512],
                                    start=(h == 0),
                                    stop=(h == HL - 1),
                                )
                            nc.vector.tensor_copy(
                                self.orow[:, n * 512:(n + 1) * 512], op_ps[:]
                            )
                            self.n += 1
                            if self.n == 4:
                                nc.sync.dma_start(
                                    OUT[jj * 512 + i * 128:jj * 512 + (i + 1) * 128, :],
                                    self.orow[:],
                                )

                        def finish(self):
                            while self.n < 4:
                                self.emit_subgroup()

                    norm_ctr = [0]

                    def emit_norm(av_ps, lacc, o_norm, h):
                        # l = allreduce_over_partitions(lacc0 + lacc1); the
                        # whole chain runs on Pool + DVE, no PE involvement
                        lsum = bcp.tile([128, 512], f32, tag="lsum")
                        nc.vector.tensor_add(lsum[:], lacc[0][:], lacc[1][:])
                        lred = bcp.tile([128, 512], f32, tag="lred")
                        nc.gpsimd.partition_all_reduce(
                            lred[:], lsum[:], channels=128,
                            reduce_op=bass.bass_isa.ReduceOp.add,
                        )
                        bc_sb = bcp.tile([128, 512], f32, tag="bcsb")
                        nc.vector.reciprocal(bc_sb[:], lred[:])
                        norm_ctr[0] += 1
                        on = onp.tile([128, 512], bf16, tag="onorm",
                                      name=f"on{_rep}_{norm_ctr[0]}_{h}")
                        nc.vector.tensor_mul(on[:], av_ps[:], bc_sb[:])
                        o_norm[h] = on

                    def attn_block(j, filler, o_prev):
                        sl = slice(j * 512, (j + 1) * 512)
                        nkb = 4 * j + 4
                        # causal trim (j>=1): diagonal chunk m only touches
                        # queries q >= 128m, so compute the [128m:512) suffix
                        # only. Diagonals run m=3..0 so the group's stop lands
                        # on a full-width matmul (m=0); j=0 keeps full width
                        # (its pos<2 lacc copies must cover all 512 cols).
                        if j >= 1:
                            order = list(range(4 * j)) + [4 * j + 3, 4 * j + 2,
                                                          4 * j + 1, 4 * j]
                        else:
                            order = list(range(nkb))
                        o_norm = [None] * HL
                        for h in range(HL):
                            pending = OProj(j - 1, h, o_prev) if o_prev else None
                            if "avp3" in psum_extra and h % 2:
                                av_ps = psum_extra["avp3"].tile(
                                    [128, 512], f32, tag="av")
                            else:
                                av_ps = avp.tile([128, 512], f32, tag="av")
                            lacc = [
                                lap.tile([128, 512], bf16, tag="lacc",
                                         name=f"lacc{_rep}_{j}_{h}_{kk}")
                                for kk in range(2)
                            ]
                            for pos, kb in enumerate(order):
                                m = kb - 4 * j
                                qs = 128 * m if (j >= 1 and m > 0) else 0
                                if "scp3" in psum_extra and pos % 2:
                                    sc_ps = psum_extra["scp3"].tile(
                                        [128, 512], f32, tag="sc")
                                else:
                                    sc_ps = scp.tile([128, 512], f32, tag="sc")
                                nc.tensor.matmul(
                                    sc_ps[:, qs:],
                                    kT[h][:, kb * 128:(kb + 1) * 128],
                                    qT[h][:, j * 512 + qs:(j + 1) * 512],
                                    start=True,
                                    stop=True,
                                )
                                ex = sbp.tile([128, 512], bf16, tag="expT")
                                nc.scalar.activation(ex[:, qs:], sc_ps[:, qs:],
                                                     EXP, scale=SCALE)
                                if m >= 0:  # diagonal: causal mask
                                    if qs > 0 or m == 0:
                                        # triangular part only spans 128 cols
                                        nc.vector.tensor_mul(
                                            ex[:, qs:qs + 128],
                                            ex[:, qs:qs + 128],
                                            masks[:, m, qs:qs + 128],
                                        )
                                    else:  # j == 0, m > 0: untrimmed
                                        nc.vector.tensor_mul(
                                            ex[:], ex[:], masks[:, m, :]
                                        )
                                nc.tensor.matmul(
                                    av_ps[:, qs:],
                                    v_sb[:, kb, h * 128:(h + 1) * 128],
                                    ex[:, qs:],
                                    start=(pos == 0),
                                    stop=(pos == nkb - 1),
                                )
                                if pos < 2:
                                    nc.vector.tensor_copy(lacc[pos][:], ex[:])
                                else:
                                    nc.vector.tensor_add(
                                        lacc[pos % 2][:, qs:],
                                        lacc[pos % 2][:, qs:],
                                        ex[:, qs:],
                                    )
                                # o_proj filler pacing: for the last block
                                # (no proj filler) spread subgroups across the
                                # long kb stream and keep one for the norm tail
                                pace = 5 if j == NB - 1 else 1
                                if pending is not None and (pos - 1) % pace == 0 and pos >= 1:
                                    pending.emit_subgroup()
                                if pos % 4 == 2 and filler:
                                    filler.pop(0)()
                            if j < NB - 1 and pending is not None:
                                pending.finish()
                            emit_norm(av_ps, lacc, o_norm, h)
                            if pending is not None:
                                pending.finish()
                            if filler:
                                filler.pop(0)()
                        for u in filler:
                            u()
                        return o_norm

                    # j=0..2 fused with projections of j+1; the proj PSUM
                    # pools live only for this span
                    with ExitStack() as pctx:
                        pps = pctx.enter_context(
                            tc.tile_pool(name="pps1", bufs=2, space="PSUM"))
                        vps = pctx.enter_context(
                            tc.tile_pool(name="vps1", bufs=2, space="PSUM"))
                        psum_extra["pps"] = pps
                        psum_extra["vps"] = vps
                        for u in proj_units(0):
                            u()
                        o_prev = None
                        for j in range(NB - 1):
                            o_prev = attn_block(j, proj_units(j + 1), o_prev)
                    del psum_extra["pps"], psum_extra["vps"]
                    # j=3 has no proj filler; deepen its pipelines with the
                    # banks freed by the projection pools
                    psum_extra["scp3"] = ctx.enter_context(
                        tc.tile_pool(name="scp3", bufs=2, space="PSUM"))
                    psum_extra["avp3"] = ctx.enter_context(
                        tc.tile_pool(name="avp3", bufs=1, space="PSUM"))
                    psum_extra["opp3"] = ctx.enter_context(
                        tc.tile_pool(name="opp3", bufs=1, space="PSUM"))
                    o_prev = attn_block(NB - 1, [], o_prev)

                    for i in range(4):
                        OProj(NB - 1, i, o_prev).finish()

    nc.compile()
    return nc


def _get_nc(repeat=1):
    key = ("nc", repeat)
    if key not in _CACHE:
        _CACHE[key] = _build(repeat=repeat)
    return _CACHE[key]


def _make_masks():
    sk = np.arange(128)[:, None]
    sq = np.arange(512)[None, :]
    m = np.stack([(sq >= sk + 128 * mm) for mm in range(4)], axis=1)
    return m.astype(ml_dtypes.bfloat16)


def _prepare_in_maps(hidden_states, cos, sin, Wq, Wk, Wv, Wo):
    bf = ml_dtypes.bfloat16
    hidden_states = np.asarray(hidden_states, dtype=np.float32)
    cos = np.asarray(cos, dtype=np.float32)
    sin = np.asarray(sin, dtype=np.float32)

    masks = _make_masks()
    e4 = ml_dtypes.float8_e4m3
    WS = 64.0  # power-of-2 fp8 weight scale, folded into cos/sin
    in_maps = []
    hsT = [np.ascontiguousarray(hidden_states[b].T).astype(bf) for b in range(B)]
    hs8T = [np.clip(np.ascontiguousarray(hidden_states[b].T), -240, 240).astype(e4)
            for b in range(B)]
    cosT = [np.ascontiguousarray(cos[b].T / WS).astype(bf) for b in range(B)]
    sinTs = []
    for b in range(B):
        s = np.ascontiguousarray(sin[b].T) / WS
        s[:64] *= -1.0
        sinTs.append(s.astype(bf))
    Wq8 = np.clip(np.asarray(Wq, dtype=np.float32) * WS, -240, 240).astype(e4)
    Wk8 = np.clip(np.asarray(Wk, dtype=np.float32) * WS, -240, 240).astype(e4)
    Wv = np.asarray(Wv, dtype=np.float32).astype(bf)
    Wo = np.asarray(Wo, dtype=np.float32).astype(bf)
    for c in range(8):
        b, g = c // 4, c % 4
        cols = slice(512 * g, 512 * (g + 1))
        in_maps.append({
            "hsT": hsT[b],
            "hs8T": hs8T[b],
            "wq": np.ascontiguousarray(Wq8[:, cols]),
            "wk": np.ascontiguousarray(Wk8[:, cols]),
            "wv": np.ascontiguousarray(Wv[:, cols]),
            "wo": np.ascontiguousarray(Wo[cols, :]),
            "cosT": cosT[b],
            "sinTs": sinTs[b],
            "masks": masks,
        })
    return in_maps


def kernel(hidden_states, cos, sin, Wq, Wk, Wv, Wo):
    nc = _get_nc()
    in_maps = _prepare_in_maps(hidden_states, cos, sin, Wq, Wk, Wv, Wo)
    res = run_bass_kernel_spmd(nc, in_maps, core_ids=list(range(8)))
    global LAST_EXEC_NS
    if res.exec_time_ns is not None:
        LAST_EXEC_NS = res.exec_time_ns
    out = np.empty((B, S, E), dtype=np.float32)
    for b in range(B):
        acc = res.results[4 * b]["out"].astype(np.float32)
        for g in range(1, 4):
            acc = acc + res.results[4 * b + g]["out"]
        out[b] = acc
    return out

